# revision 1
# baseline (speedup 1.0000x reference)
"""Pairwise L2-distance kernel (retrieval_knn) for 8x Trainium2 NeuronCores.

Computes Z = beta - sqrt(max(||x||^2 + ||y||^2 - 2 X@Y, 0)) for
X:(8192,256) f32, Y:(256,8192) f32, beta:(1,) f32 -> Z:(8192,8192) f32.

Sharding: X row-wise across 8 cores (1024 rows each); Y and beta replicated.
Each core computes a (1024, 8192) slab of Z; the host concatenates slabs.

Per-core algorithm:
  - cross term via PE matmul in bf16 with X scaled by -2 at convert time
    (exact power-of-two scaling), K=256 split into 2 chunks of 128.
  - ||y||^2 injected into the same PSUM accumulation as one extra fp16
    contraction row (ones (x) y2_fp16); fp16 keeps y2's absolute error
    ~0.12 on values ~512 (vs ~1-2 for bf16).
  - ||x||^2 (exact fp32) added via the per-partition bias of the fused
    ScalarE Sqrt activation reading PSUM.
  - z = beta - d in a single VectorE tensor_scalar pass: (d * -1) + beta.
"""

from contextlib import ExitStack

import numpy as np

import concourse.bacc as bacc
import concourse.mybir as mybir
import concourse.tile as tile
from concourse.bass_utils import run_bass_kernel_spmd
from concourse.masks import make_identity

N_CORES = 8
N_ROW, RANK, N_COL = 8192, 256, 8192
ROWS_PER_CORE = N_ROW // N_CORES  # 1024

P = 128      # partitions
FN = 512     # matmul free dim / PSUM bank (fp32)

f32 = mybir.dt.float32
bf16 = mybir.dt.bfloat16
f16 = mybir.dt.float16

AF = mybir.ActivationFunctionType
ALU = mybir.AluOpType


def build_l2_kernel(rows=ROWS_PER_CORE, rank=RANK, ncol=N_COL, n_cores=N_CORES,
                    out_bufs=8, psum_bufs=6,
                    use_e_row=True, use_beta_ap=True, use_x_side=True,
                    use_y_side=True, use_main=True,
                    use_x2=True, use_xT=True):
    """Build the per-core SPMD Bass program. Returns the compiled Bacc."""
    assert rows % P == 0 and rank % P == 0 and ncol % FN == 0
    mt = rows // P          # m-tiles (8)
    kc = rank // P          # k-chunks (2)
    nt = ncol // FN         # n-tiles (16)

    nc = bacc.Bacc("TRN2", target_bir_lowering=False, debug=False,
                   num_devices=n_cores)

    xs_d = nc.dram_tensor("Xs", [rows, rank], f32, kind="ExternalInput")
    y_d = nc.dram_tensor("Y", [rank, ncol], f32, kind="ExternalInput")
    beta_d = nc.dram_tensor("beta", [1, 1], f32, kind="ExternalInput")
    # Z stored as [mt, nt, 128, 512] tile blocks -> every DMA store is one
    # fully contiguous 256KB burst. Host reassembles to [rows, ncol].
    z_d = nc.dram_tensor("Z", [mt * nt * P, FN], f32, kind="ExternalOutput")

    with tile.TileContext(nc) as tc, ExitStack() as ctx:
        cpool = ctx.enter_context(tc.tile_pool(name="const", bufs=1))
        ypool = ctx.enter_context(tc.tile_pool(name="ybig", bufs=1))
        yf_pool = ctx.enter_context(tc.tile_pool(name="yf", bufs=6))
        ysq_pool = ctx.enter_context(tc.tile_pool(name="ysq", bufs=6))
        setup_psum = ExitStack()
        tp_psum = setup_psum.enter_context(
            tc.tile_pool(name="tpp", bufs=2, space="PSUM"))
        y2_psum = setup_psum.enter_context(
            tc.tile_pool(name="y2p", bufs=2, space="PSUM"))
        dpool = ctx.enter_context(tc.tile_pool(name="d", bufs=out_bufs))

        # ---- constants ----
        identity = cpool.tile([P, P], f32)
        make_identity(nc, identity[:])
        ones_row = cpool.tile([1, P], f16)       # lhsT of the y2-row matmul
        nc.gpsimd.memset(ones_row[:], 1.0)
        ones_col = cpool.tile([P, 1], bf16)      # lhsT of the y2 column-reduce
        nc.gpsimd.memset(ones_col[:], 1.0)
        if use_beta_ap:
            beta_b = cpool.tile([P, 1], f32)
            b11 = cpool.tile([1, 1], f32)
            nc.sync.dma_start(b11[:], beta_d.ap()[:])
            nc.gpsimd.partition_broadcast(beta_b[:], b11[:])
        else:
            beta_b = None

        # ---- X side: load slab, x2, transposed -2X in bf16 ----
        xs_sb = cpool.tile([P, mt, rank], f32)
        nc.sync.dma_start(
            xs_sb[:], xs_d.ap().rearrange("(t p) k -> p t k", p=P))

        x2 = cpool.tile([P, mt], f32)
        xsq = cpool.tile([P, rank], f32)
        xbT = [cpool.tile([P, rows], bf16, name=f"xbT{c}", tag=f"xbT{c}")
               for c in range(kc)]
        for m in range(mt if use_x_side else 0):
            if use_x2:
                nc.vector.tensor_tensor(
                    xsq[:], xs_sb[:, m, :], xs_sb[:, m, :], op=ALU.mult)
                nc.vector.reduce_sum(
                    x2[:, m : m + 1], xsq[:], axis=mybir.AxisListType.X)
            for c in range(kc if use_xT else 0):
                pt = tp_psum.tile([P, P], f32)
                nc.tensor.transpose(
                    pt[:], xs_sb[:, m, c * P : (c + 1) * P], identity[:])
                nc.scalar.activation(
                    xbT[c][:, m * P : (m + 1) * P], pt[:],
                    AF.Copy, scale=-2.0)

        # ---- Y side: stream pieces, convert to bf16, y2 -> fp16 row ----
        yb = [ypool.tile([P, ncol], bf16, name=f"yb{c}", tag=f"yb{c}")
              for c in range(kc)]
        e_row = cpool.tile([1, ncol], f16)
        for j in range(nt if use_y_side else 0):
            y2ps = y2_psum.tile([1, FN], f32)
            for c in range(kc):
                yf = yf_pool.tile([P, FN], f32)
                nc.sync.dma_start(
                    yf[:], y_d.ap()[c * P : (c + 1) * P,
                                    j * FN : (j + 1) * FN])
                nc.vector.tensor_copy(yb[c][:, j * FN : (j + 1) * FN], yf[:])
                ysq = ysq_pool.tile([P, FN], bf16)
                nc.scalar.activation(ysq[:], yf[:], AF.Square)
                nc.tensor.matmul(
                    y2ps[:], ones_col[:], ysq[:],
                    start=(c == 0), stop=(c == kc - 1))
            if use_e_row:
                nc.scalar.activation(
                    e_row[:, j * FN : (j + 1) * FN], y2ps[:], AF.Copy)

        # ---- main loop ----
        # setup PSUM pools released here -> main matmuls get 6 banks
        setup_psum.close()
        mm_psum = ctx.enter_context(
            tc.tile_pool(name="mmp", bufs=psum_bufs, space="PSUM"))
        for m in range(mt if (use_main and use_x_side and use_y_side) else 0):
            for j in range(nt):
                ps = mm_psum.tile([P, FN], f32)
                for c in range(kc):
                    nc.tensor.matmul(
                        ps[:], xbT[c][:, m * P : (m + 1) * P],
                        yb[c][:, j * FN : (j + 1) * FN],
                        start=(c == 0), stop=(not use_e_row and c == kc - 1))
                if use_e_row:
                    nc.tensor.matmul(
                        ps[:], ones_row[:],
                        e_row[:, j * FN : (j + 1) * FN],
                        start=False, stop=True)
                d = dpool.tile([P, FN], f32)
                nc.scalar.activation(
                    d[:], ps[:], AF.Sqrt, bias=x2[:, m : m + 1])
                nc.vector.tensor_scalar(
                    d[:], d[:], -1.0,
                    beta_b[:] if use_beta_ap else 0.0, ALU.mult, ALU.add)
                blk = (m * nt + j) * P
                nc.sync.dma_start(z_d.ap()[blk : blk + P, :], d[:])

    nc.compile()
    return nc


_CACHED = {}


def _get_nc():
    if "nc" not in _CACHED:
        _CACHED["nc"] = build_l2_kernel()
    return _CACHED["nc"]


def kernel(X, Y, beta):
    X = np.ascontiguousarray(np.asarray(X, dtype=np.float32))
    Y = np.ascontiguousarray(np.asarray(Y, dtype=np.float32))
    beta = np.asarray(beta, dtype=np.float32).reshape(1, 1)
    assert X.shape == (N_ROW, RANK) and Y.shape == (RANK, N_COL)

    nc = _get_nc()
    in_maps = [
        {"Xs": X[c * ROWS_PER_CORE : (c + 1) * ROWS_PER_CORE], "Y": Y,
         "beta": beta}
        for c in range(N_CORES)
    ]
    res = run_bass_kernel_spmd(nc, in_maps, core_ids=list(range(N_CORES)))
    mt, nt = ROWS_PER_CORE // P, N_COL // FN
    slabs = [
        res.results[c]["Z"].reshape(mt, nt, P, FN)
        .transpose(0, 2, 1, 3).reshape(ROWS_PER_CORE, N_COL)
        for c in range(N_CORES)
    ]
    return np.ascontiguousarray(np.concatenate(slabs, axis=0))



# revision 2
# speedup vs baseline: 6.3422x; 6.3422x over previous
"""Pairwise L2-distance kernel (retrieval_knn) for 8x Trainium2 NeuronCores.

Z = beta - sqrt(max(||x||^2 + ||y||^2 - 2 X@Y, 0)),
X:(8192,256) f32, Y:(256,8192) f32, beta:(1,) -> Z:(8192,8192) f32.
X row-sharded over 8 cores; Y/beta replicated.

Structure (per core):
  - fp8e4 DoubleRow cross matmuls: K=256 in one PE instruction per
    512-wide PSUM quarter; ||y||^2 added via one fp16 ones-row matmul.
  - software-pipelined chunk loop: {DMA Y chunk -> fp8 convert (DVE),
    squares (Pool), y2 column-reduce (PE, into an mm-psum scratch tile),
    e_row copy (DVE)} immediately followed by that chunk's 8 m-groups:
    {matmuls -> ScalarE sqrt(ps + x2) -> f16, DVE (d*-1)+beta in f16,
    256KB f16 block store}.
  - ScalarE runs only x-side prep + the 32 sqrt groups (2048 wide, 4 PSUM
    banks, double buffered); everything else lives on DVE/Pool/PE.
  - output f16, upcast to f32 on the host.
"""

from contextlib import ExitStack

import numpy as np

import concourse.bacc as bacc
import concourse.mybir as mybir
import concourse.tile as tile
from concourse.bass_utils import run_bass_kernel_spmd
from concourse.masks import make_identity

N_CORES = 8
N_ROW, RANK, N_COL = 8192, 256, 8192
ROWS_PER_CORE = N_ROW // N_CORES  # 1024

P = 128      # partitions
FN = 512     # matmul free dim / PSUM bank (fp32)
GW = 2048    # ACT/DVE group width (4 PSUM banks) and Y chunk width

f32 = mybir.dt.float32
bf16 = mybir.dt.bfloat16
f16 = mybir.dt.float16
f8 = mybir.dt.float8e4

AF = mybir.ActivationFunctionType
ALU = mybir.AluOpType
DR = mybir.MatmulPerfMode.DoubleRow


def build_l2_kernel(rows=ROWS_PER_CORE, rank=RANK, ncol=N_COL, n_cores=N_CORES,
                    use_fp8=True, gw=GW, d_bufs=6, psum_bufs=2, yf_bufs=2, post_at=4, store_w=2,
                    x2_on="act", scales_on="act", split_sq0=True):
    """Build the per-core SPMD Bass program. Returns the compiled Bacc."""
    kc = rank // P          # k-chunks (2)
    mt = rows // P          # m-tiles (8)
    ng = ncol // gw         # chunks == main-loop column groups (4)
    qn = gw // FN           # 512-quarters per group (4)
    assert rows % P == 0 and rank == 2 * P and ncol % gw == 0 and gw % FN == 0

    nc = bacc.Bacc("TRN2", target_bir_lowering=False, debug=False,
                   num_devices=n_cores)

    xs_d = nc.dram_tensor("Xs", [rows, rank], f32, kind="ExternalInput")
    y_d = nc.dram_tensor("Y", [rank, ncol], f32, kind="ExternalInput")
    beta_d = nc.dram_tensor("beta", [1, 1], f32, kind="ExternalInput")
    # Z stored as [m, g, 128, gw] f16 blocks; host reassembles + upcasts.
    z_d = nc.dram_tensor("Z", [mt * ng * P, gw], f16, kind="ExternalOutput")

    with tile.TileContext(nc) as tc, ExitStack() as ctx:
        cpool = ctx.enter_context(tc.tile_pool(name="const", bufs=1))
        yf_pool = ctx.enter_context(tc.tile_pool(name="yf", bufs=yf_bufs))
        ysq_pool = ctx.enter_context(tc.tile_pool(name="ysq", bufs=2))
        dpool = ctx.enter_context(tc.tile_pool(name="d", bufs=d_bufs))

        # ---- constants ----
        identity = cpool.tile([P, P], f32)
        make_identity(nc, identity[:])
        ones_row = cpool.tile([1, P], f16)       # lhsT of the y2-row matmul
        nc.gpsimd.memset(ones_row[:], 1.0)
        ones_col = cpool.tile([P, 1], bf16)      # lhsT of the y2 column-reduce
        nc.gpsimd.memset(ones_col[:], 1.0)
        beta_b = cpool.tile([P, 1], f32)
        b11 = cpool.tile([1, 1], f32)
        nc.sync.dma_start(b11[:], beta_d.ap()[:])
        nc.gpsimd.partition_broadcast(beta_b[:], b11[:])

        xdt = f8 if use_fp8 else bf16
        x2 = cpool.tile([P, mt], f32)
        xsq = cpool.tile([P, rank], f32)
        xT8 = cpool.tile([P, kc, rows], xdt)
        y8 = cpool.tile([P, kc, ncol], xdt)
        e_row = cpool.tile([1, ncol], f16)
        xs_sb = cpool.tile([P, mt, rank], f32)
        y_ap = y_d.ap().rearrange("(c p) n -> p c n", p=P)

        # ---- loads: Y chunk 0 first (it gates the first e_row chain),
        # then X; later Y chunks prefetched inside the pipeline.
        yf_tiles = {}

        def load_chunk(i):
            yfi = yf_pool.tile([P, kc, gw], f32, name="yf", tag="yf")
            nc.sync.dma_start(yfi[:], y_ap[:, :, i * gw : (i + 1) * gw])
            yf_tiles[i] = yfi

        nc.sync.dma_start(
            xs_sb[:], xs_d.ap().rearrange("(t p) k -> p t k", p=P))
        load_chunk(0)

        # ---- X side ----
        tp_psum_ctx = ExitStack()
        tp_psum = tp_psum_ctx.enter_context(
            tc.tile_pool(name="tpp", bufs=2, space="PSUM"))
        # 4 transposes (c-major m-pairs) per PSUM bank -> one batched
        # -2x fp8 convert per bank (issued BEFORE the x2 squares so xT8 is
        # ready for the first DR matmul as early as possible)
        for mp in range(mt // 2):
            m0 = mp * 2
            pt = tp_psum.tile([P, 2, 2, P], f32)
            for c in range(kc):
                for dm in range(2):
                    nc.tensor.transpose(
                        pt[:, c, dm], xs_sb[:, m0 + dm, c * P : (c + 1) * P],
                        identity[:])
            dst = xT8[:, :, m0 * P : (m0 + 2) * P]
            if scales_on == "act":
                nc.scalar.activation(dst, pt[:], AF.Copy, scale=-2.0)
            elif scales_on == "vec":
                nc.vector.tensor_scalar(dst, pt[:], -2.0, None, op0=ALU.mult)
            else:
                nc.gpsimd.tensor_scalar(dst, pt[:], -2.0, None, op0=ALU.mult)
        tp_psum_ctx.close()
        if x2_on == "act":
            for m in range(mt):
                nc.scalar.activation(
                    xsq[:, 0:256], xs_sb[:, m, :], AF.Square,
                    accum_out=x2[:, m : m + 1])

        mm_psum = ctx.enter_context(
            tc.tile_pool(name="mmp", bufs=psum_bufs, space="PSUM"))

        ysq_tiles = {}

        def process_pre(i):
            """fp8 convert + squares for Y chunk i (DVE/Pool only)."""
            j0 = i * gw
            yfi = yf_tiles.pop(i)
            nc.vector.tensor_copy(y8[:, :, j0 : j0 + gw], yfi[:])
            ysq = ysq_pool.tile([P, kc, gw], bf16)
            # squares split into column halves: Pool does b01, DVE b23
            h = gw // 2
            nc.gpsimd.tensor_tensor(
                ysq[:, :, 0:h], yfi[:, :, 0:h], yfi[:, :, 0:h], op=ALU.mult)
            nc.vector.tensor_tensor(
                ysq[:, :, h:gw], yfi[:, :, h:gw], yfi[:, :, h:gw],
                op=ALU.mult)
            ysq_tiles[i] = ysq

        def process_post(i):
            """y2 column reduce (PE) + e_row copies (Pool) for chunk i."""
            j0 = i * gw
            ysq = ysq_tiles.pop(i)
            y2scr = mm_psum.tile([P, gw], f32, name="y2scr", tag="mm")
            for b in range(qn):
                sl = y2scr[0:1, b * FN : (b + 1) * FN]
                for c in range(kc):
                    nc.tensor.matmul(
                        sl, ones_col[:], ysq[:, c, b * FN : (b + 1) * FN],
                        start=(c == 0), stop=(c == kc - 1))
                nc.vector.tensor_copy(
                    e_row[:, j0 + b * FN : j0 + (b + 1) * FN], sl)

        # ---- software-pipelined chunk loop. chunk i+1's DMA + DVE/Pool
        # processing are issued before octet i (they don't touch the PE),
        # but its y2-reduce matmuls go AFTER octet i so the in-order PE
        # queue never stalls on not-yet-loaded Y data.
        process_pre(0)
        process_post(0)
        for i in range(ng):
            j0 = i * gw
            if i + 1 < ng:
                load_chunk(i + 1)
                process_pre(i + 1)
            for m in range(mt):
                ps = mm_psum.tile([P, gw], f32, name="ps", tag="mm")
                for q in range(qn):
                    jq = j0 + q * FN
                    sl = ps[:, q * FN : (q + 1) * FN]
                    if use_fp8:
                        nc.tensor.matmul(
                            sl, xT8[:, :, m * P : (m + 1) * P],
                            y8[:, :, jq : jq + FN],
                            start=True, stop=False, perf_mode=DR)
                    else:
                        for c in range(kc):
                            nc.tensor.matmul(
                                sl, xT8[:, c, m * P : (m + 1) * P],
                                y8[:, c, jq : jq + FN],
                                start=(c == 0), stop=False)
                    nc.tensor.matmul(
                        sl, ones_row[:], e_row[:, jq : jq + FN],
                        start=False, stop=True)
                d = dpool.tile([P, gw], f16)
                nc.scalar.activation(
                    d[:], ps[:], AF.Sqrt, bias=x2[:, m : m + 1])
                nc.vector.tensor_scalar(
                    d[:], d[:], -1.0, beta_b[:], op0=ALU.mult, op1=ALU.add)
                blk = (m * ng + i) * P
                nc.sync.dma_start(z_d.ap()[blk : blk + P, :], d[:])
                if m == min(post_at, mt - 1) and i + 1 < ng:
                    process_post(i + 1)
                del d

    nc.compile()
    return nc


_CACHED = {}


def _get_nc():
    if "nc" not in _CACHED:
        _CACHED["nc"] = build_l2_kernel()
    return _CACHED["nc"]


def kernel(X, Y, beta):
    X = np.ascontiguousarray(np.asarray(X, dtype=np.float32))
    Y = np.ascontiguousarray(np.asarray(Y, dtype=np.float32))
    beta = np.asarray(beta, dtype=np.float32).reshape(1, 1)
    assert X.shape == (N_ROW, RANK) and Y.shape == (RANK, N_COL)

    nc = _get_nc()
    in_maps = [
        {"Xs": X[c * ROWS_PER_CORE : (c + 1) * ROWS_PER_CORE], "Y": Y,
         "beta": beta}
        for c in range(N_CORES)
    ]
    res = run_bass_kernel_spmd(nc, in_maps, core_ids=list(range(N_CORES)))
    mt, ng = ROWS_PER_CORE // P, N_COL // GW
    out = np.empty((N_ROW, N_COL), dtype=np.float32)
    for c in range(N_CORES):
        slab = res.results[c]["Z"].reshape(mt, ng, P, GW)
        slab = slab.transpose(0, 2, 1, 3).reshape(ROWS_PER_CORE, N_COL)
        out[c * ROWS_PER_CORE : (c + 1) * ROWS_PER_CORE] = slab
    return out


# revision 3
# speedup vs baseline: 6.4362x; 1.0148x over previous
"""Pairwise L2-distance kernel (retrieval_knn) for 8x Trainium2 NeuronCores.

Z = beta - sqrt(max(||x||^2 + ||y||^2 - 2 X@Y, 0)),
X:(8192,256) f32, Y:(256,8192) f32, beta:(1,) -> Z:(8192,8192) f32.
X row-sharded over 8 cores; Y/beta replicated.

Structure (per core):
  - fp8e4 DoubleRow cross matmuls: K=256 in one PE instruction per
    512-wide PSUM quarter; ||y||^2 added via one fp16 ones-row matmul.
  - software-pipelined chunk loop: {DMA Y chunk -> fp8 convert (DVE),
    squares (Pool), y2 column-reduce (PE, into an mm-psum scratch tile),
    e_row copy (DVE)} immediately followed by that chunk's 8 m-groups:
    {matmuls -> ScalarE sqrt(ps + x2) -> f16, DVE (d*-1)+beta in f16,
    256KB f16 block store}.
  - ScalarE runs only x-side prep + the 32 sqrt groups (2048 wide, 4 PSUM
    banks, double buffered); everything else lives on DVE/Pool/PE.
  - output f16, upcast to f32 on the host.
"""

from contextlib import ExitStack

import numpy as np

import concourse.bacc as bacc
import concourse.mybir as mybir
import concourse.tile as tile
from concourse.bass_utils import run_bass_kernel_spmd
from concourse.masks import make_identity

N_CORES = 8
N_ROW, RANK, N_COL = 8192, 256, 8192
ROWS_PER_CORE = N_ROW // N_CORES  # 1024

P = 128      # partitions
FN = 512     # matmul free dim / PSUM bank (fp32)
GW = 2048    # ACT/DVE group width (4 PSUM banks) and Y chunk width

f32 = mybir.dt.float32
bf16 = mybir.dt.bfloat16
f16 = mybir.dt.float16
f8 = mybir.dt.float8e4

AF = mybir.ActivationFunctionType
ALU = mybir.AluOpType
DR = mybir.MatmulPerfMode.DoubleRow


def build_l2_kernel(rows=ROWS_PER_CORE, rank=RANK, ncol=N_COL, n_cores=N_CORES,
                    use_fp8=True, gw=GW, d_bufs=8, psum_bufs=2, yf_bufs=2, post_at=4,
                    x2_on="act", scales_on="act", split_sq0=True):
    """Build the per-core SPMD Bass program. Returns the compiled Bacc."""
    kc = rank // P          # k-chunks (2)
    mt = rows // P          # m-tiles (8)
    ng = ncol // gw         # chunks == main-loop column groups (4)
    qn = gw // FN           # 512-quarters per group (4)
    assert rows % P == 0 and rank == 2 * P and ncol % gw == 0 and gw % FN == 0

    nc = bacc.Bacc("TRN2", target_bir_lowering=False, debug=False,
                   num_devices=n_cores)

    xs_d = nc.dram_tensor("Xs", [rows, rank], f32, kind="ExternalInput")
    y_d = nc.dram_tensor("Y", [rank, ncol], f32, kind="ExternalInput")
    beta_d = nc.dram_tensor("beta", [1, 1], f32, kind="ExternalInput")
    # Z stored as [m, g, 128, gw] f16 blocks; host reassembles + upcasts.
    z_d = nc.dram_tensor("Z", [mt * ng * P, gw], f16, kind="ExternalOutput")

    with tile.TileContext(nc) as tc, ExitStack() as ctx:
        cpool = ctx.enter_context(tc.tile_pool(name="const", bufs=1))
        yf_pool = ctx.enter_context(tc.tile_pool(name="yf", bufs=yf_bufs))
        ysq_pool = ctx.enter_context(tc.tile_pool(name="ysq", bufs=2))
        dpool = ctx.enter_context(tc.tile_pool(name="d", bufs=d_bufs))

        # ---- constants ----
        identity = cpool.tile([P, P], f32)
        make_identity(nc, identity[:])
        warm = cpool.tile([1, 1], f32)
        nc.scalar.activation(warm[:], identity[0:1, 0:1], AF.Sqrt)
        ones_row = cpool.tile([1, P], f16)       # lhsT of the y2-row matmul
        nc.gpsimd.memset(ones_row[:], 1.0)
        ones_col = cpool.tile([P, 1], bf16)      # lhsT of the y2 column-reduce
        nc.gpsimd.memset(ones_col[:], 1.0)
        beta_b = cpool.tile([P, 1], f32)
        b11 = cpool.tile([1, 1], f32)
        nc.sync.dma_start(b11[:], beta_d.ap()[:])
        nc.gpsimd.partition_broadcast(beta_b[:], b11[:])

        xdt = f8 if use_fp8 else bf16
        x2 = cpool.tile([P, mt], f32)
        xsq = cpool.tile([P, rank], f32)
        xT8 = cpool.tile([P, kc, rows], xdt)
        y8 = cpool.tile([P, kc, ncol], xdt)
        e_row = cpool.tile([1, ncol], f16)
        xs_sb = cpool.tile([P, mt, rank], f32)
        y_ap = y_d.ap().rearrange("(c p) n -> p c n", p=P)

        # ---- loads: Y chunk 0 first (it gates the first e_row chain),
        # then X; later Y chunks prefetched inside the pipeline.
        yf_tiles = {}

        def load_chunk(i):
            yfi = yf_pool.tile([P, kc, gw], f32, name="yf", tag="yf")
            nc.sync.dma_start(yfi[:], y_ap[:, :, i * gw : (i + 1) * gw])
            yf_tiles[i] = yfi

        hw_ = gw // 2
        yf0h = []
        for h in range(2):
            t = yf_pool.tile([P, kc, hw_], f32, name="yfh", tag="yfh")
            nc.sync.dma_start(t[:], y_ap[:, :, h * hw_ : (h + 1) * hw_])
            yf0h.append(t)
        nc.sync.dma_start(
            xs_sb[:], xs_d.ap().rearrange("(t p) k -> p t k", p=P))

        # ---- X side ----
        tp_psum_ctx = ExitStack()
        tp_psum = tp_psum_ctx.enter_context(
            tc.tile_pool(name="tpp", bufs=2, space="PSUM"))
        # 4 transposes (c-major m-pairs) per PSUM bank -> one batched
        # -2x fp8 convert per bank (issued BEFORE the x2 squares so xT8 is
        # ready for the first DR matmul as early as possible)
        for mp in range(mt // 2):
            m0 = mp * 2
            pt = tp_psum.tile([P, 2, 2, P], f32)
            for c in range(kc):
                for dm in range(2):
                    nc.tensor.transpose(
                        pt[:, c, dm], xs_sb[:, m0 + dm, c * P : (c + 1) * P],
                        identity[:])
            dst = xT8[:, :, m0 * P : (m0 + 2) * P]
            if scales_on == "act":
                nc.scalar.activation(dst, pt[:], AF.Copy, scale=-2.0)
            elif scales_on == "vec":
                nc.vector.tensor_scalar(dst, pt[:], -2.0, None, op0=ALU.mult)
            else:
                nc.gpsimd.tensor_scalar(dst, pt[:], -2.0, None, op0=ALU.mult)
        tp_psum_ctx.close()
        if x2_on == "act":
            for m in range(mt):
                nc.scalar.activation(
                    xsq[:, 0:256], xs_sb[:, m, :], AF.Square,
                    accum_out=x2[:, m : m + 1])

        mm_psum = ctx.enter_context(
            tc.tile_pool(name="mmp", bufs=psum_bufs, space="PSUM"))

        ysq_tiles = {}

        def process_pre(i):
            """fp8 convert + squares for Y chunk i (DVE/Pool only)."""
            j0 = i * gw
            yfi = yf_tiles.pop(i)
            nc.vector.tensor_copy(y8[:, :, j0 : j0 + gw], yfi[:])
            ysq = ysq_pool.tile([P, kc, gw], bf16)
            # squares split into column halves: Pool does b01, DVE b23
            h = gw // 2
            nc.gpsimd.tensor_tensor(
                ysq[:, :, 0:h], yfi[:, :, 0:h], yfi[:, :, 0:h], op=ALU.mult)
            nc.vector.tensor_tensor(
                ysq[:, :, h:gw], yfi[:, :, h:gw], yfi[:, :, h:gw],
                op=ALU.mult)
            ysq_tiles[i] = ysq

        def process_post(i):
            """y2 column reduce (PE) + e_row copies (Pool) for chunk i."""
            j0 = i * gw
            ysq = ysq_tiles.pop(i)
            y2scr = mm_psum.tile([P, gw], f32, name="y2scr", tag="mm")
            for b in range(qn):
                sl = y2scr[0:1, b * FN : (b + 1) * FN]
                for c in range(kc):
                    nc.tensor.matmul(
                        sl, ones_col[:], ysq[:, c, b * FN : (b + 1) * FN],
                        start=(c == 0), stop=(c == kc - 1))
                nc.vector.tensor_copy(
                    e_row[:, j0 + b * FN : j0 + (b + 1) * FN], sl)

        # ---- software-pipelined chunk loop. chunk i+1's DMA + DVE/Pool
        # processing are issued before octet i (they don't touch the PE),
        # but its y2-reduce matmuls go AFTER octet i so the in-order PE
        # queue never stalls on not-yet-loaded Y data.
        # chunk 0 arrives as two 1024-wide halves -> first e_row sooner
        ysq0 = ysq_pool.tile([P, kc, gw], bf16, name="ysq", tag="ysq")
        y2scr0 = mm_psum.tile([P, gw], f32, name="y2scr", tag="mm")
        q_ = hw_ // 2
        for h in range(2):
            t = yf0h[h]
            nc.vector.tensor_copy(y8[:, :, h * hw_ : (h + 1) * hw_], t[:])
            nc.gpsimd.tensor_tensor(
                ysq0[:, :, h * hw_ : h * hw_ + q_],
                t[:, :, 0:q_], t[:, :, 0:q_], op=ALU.mult)
            nc.vector.tensor_tensor(
                ysq0[:, :, h * hw_ + q_ : (h + 1) * hw_],
                t[:, :, q_:hw_], t[:, :, q_:hw_], op=ALU.mult)
            for b in ((0, 1) if h == 0 else (2, 3)):
                sl = y2scr0[0:1, b * FN : (b + 1) * FN]
                for c in range(kc):
                    nc.tensor.matmul(
                        sl, ones_col[:], ysq0[:, c, b * FN : (b + 1) * FN],
                        start=(c == 0), stop=(c == kc - 1))
                nc.vector.tensor_copy(
                    e_row[:, b * FN : (b + 1) * FN], sl)
        for i in range(ng):
            j0 = i * gw
            if i + 1 < ng:
                load_chunk(i + 1)
                process_pre(i + 1)
            for m in range(mt):
                ps = mm_psum.tile([P, gw], f32, name="ps", tag="mm")
                for q in range(qn):
                    jq = j0 + q * FN
                    sl = ps[:, q * FN : (q + 1) * FN]
                    if use_fp8:
                        nc.tensor.matmul(
                            sl, xT8[:, :, m * P : (m + 1) * P],
                            y8[:, :, jq : jq + FN],
                            start=True, stop=False, perf_mode=DR)
                    else:
                        for c in range(kc):
                            nc.tensor.matmul(
                                sl, xT8[:, c, m * P : (m + 1) * P],
                                y8[:, c, jq : jq + FN],
                                start=(c == 0), stop=False)
                    nc.tensor.matmul(
                        sl, ones_row[:], e_row[:, jq : jq + FN],
                        start=False, stop=True)
                d = dpool.tile([P, gw], f16)
                nc.scalar.activation(
                    d[:], ps[:], AF.Sqrt, bias=x2[:, m : m + 1])
                nc.vector.tensor_scalar(
                    d[:], d[:], -1.0, beta_b[:], op0=ALU.mult, op1=ALU.add)
                blk = (m * ng + i) * P
                nc.sync.dma_start(z_d.ap()[blk : blk + P, :], d[:])
                if m == min(post_at, mt - 1) and i + 1 < ng:
                    process_post(i + 1)
                del d

    nc.compile()
    return nc


_CACHED = {}


def _get_nc():
    if "nc" not in _CACHED:
        _CACHED["nc"] = build_l2_kernel()
    return _CACHED["nc"]


def kernel(X, Y, beta):
    X = np.ascontiguousarray(np.asarray(X, dtype=np.float32))
    Y = np.ascontiguousarray(np.asarray(Y, dtype=np.float32))
    beta = np.asarray(beta, dtype=np.float32).reshape(1, 1)
    assert X.shape == (N_ROW, RANK) and Y.shape == (RANK, N_COL)

    nc = _get_nc()
    in_maps = [
        {"Xs": X[c * ROWS_PER_CORE : (c + 1) * ROWS_PER_CORE], "Y": Y,
         "beta": beta}
        for c in range(N_CORES)
    ]
    res = run_bass_kernel_spmd(nc, in_maps, core_ids=list(range(N_CORES)))
    mt, ng = ROWS_PER_CORE // P, N_COL // GW
    out = np.empty((N_ROW, N_COL), dtype=np.float32)
    for c in range(N_CORES):
        slab = res.results[c]["Z"].reshape(mt, ng, P, GW)
        slab = slab.transpose(0, 2, 1, 3).reshape(ROWS_PER_CORE, N_COL)
        out[c * ROWS_PER_CORE : (c + 1) * ROWS_PER_CORE] = slab
    return out


# revision 4
# speedup vs baseline: 6.6649x; 1.0355x over previous
"""Pairwise L2-distance kernel (retrieval_knn) for 8x Trainium2 NeuronCores.

Z = beta - sqrt(max(||x||^2 + ||y||^2 - 2 X@Y, 0)),
X:(8192,256) f32, Y:(256,8192) f32, beta:(1,) -> Z:(8192,8192) f32.
X row-sharded over 8 cores; Y/beta replicated.

Structure (per core):
  - fp8e4 DoubleRow cross matmuls: K=256 in one PE instruction per
    512-wide PSUM quarter; ||y||^2 added via one fp16 ones-row matmul.
  - software-pipelined chunk loop: {DMA Y chunk -> fp8 convert (DVE),
    squares (Pool), y2 column-reduce (PE, into an mm-psum scratch tile),
    e_row copy (DVE)} immediately followed by that chunk's 8 m-groups:
    {matmuls -> ScalarE sqrt(ps + x2) -> f16, DVE (d*-1)+beta in f16,
    256KB f16 block store}.
  - ScalarE runs only x-side prep + the 32 sqrt groups (2048 wide, 4 PSUM
    banks, double buffered); everything else lives on DVE/Pool/PE.
  - output f16, upcast to f32 on the host.
"""

from contextlib import ExitStack

import numpy as np

import concourse.bacc as bacc
import concourse.mybir as mybir
import concourse.tile as tile
from concourse.bass_utils import run_bass_kernel_spmd
from concourse.masks import make_identity

N_CORES = 8
N_ROW, RANK, N_COL = 8192, 256, 8192
ROWS_PER_CORE = N_ROW // N_CORES  # 1024

P = 128      # partitions
FN = 512     # matmul free dim / PSUM bank (fp32)
GW = 2048    # ACT/DVE group width (4 PSUM banks) and Y chunk width

f32 = mybir.dt.float32
bf16 = mybir.dt.bfloat16
f16 = mybir.dt.float16
f8 = mybir.dt.float8e4

AF = mybir.ActivationFunctionType
ALU = mybir.AluOpType
DR = mybir.MatmulPerfMode.DoubleRow


def build_l2_kernel(rows=ROWS_PER_CORE, rank=RANK, ncol=N_COL, n_cores=N_CORES,
                    use_fp8=True, gw=GW, d_bufs=8, psum_bufs=2, yf_bufs=2, post_at=3,
                    x2_on="act", scales_on="act", split_sq0=True):
    """Build the per-core SPMD Bass program. Returns the compiled Bacc."""
    kc = rank // P          # k-chunks (2)
    mt = rows // P          # m-tiles (8)
    ng = ncol // gw         # chunks == main-loop column groups (4)
    qn = gw // FN           # 512-quarters per group (4)
    assert rows % P == 0 and rank == 2 * P and ncol % gw == 0 and gw % FN == 0

    nc = bacc.Bacc("TRN2", target_bir_lowering=False, debug=False,
                   num_devices=n_cores)

    xs_d = nc.dram_tensor("Xs", [rows, rank], f32, kind="ExternalInput")
    y_d = nc.dram_tensor("Y", [rank, ncol], f32, kind="ExternalInput")
    beta_d = nc.dram_tensor("beta", [1, 1], f32, kind="ExternalInput")
    # Z stored as [m, g, 128, gw] f16 blocks; host reassembles + upcasts.
    z_d = nc.dram_tensor("Z", [mt * ng * P, gw], f16, kind="ExternalOutput")

    with tile.TileContext(nc) as tc, ExitStack() as ctx:
        cpool = ctx.enter_context(tc.tile_pool(name="const", bufs=1))
        yf_pool = ctx.enter_context(tc.tile_pool(name="yf", bufs=yf_bufs))
        ysq_pool = ctx.enter_context(tc.tile_pool(name="ysq", bufs=2))
        dpool = ctx.enter_context(tc.tile_pool(name="d", bufs=d_bufs))

        # ---- constants ----
        identity = cpool.tile([P, P], f32)
        make_identity(nc, identity[:])
        warm = cpool.tile([1, 1], f32)
        nc.scalar.activation(warm[:], identity[0:1, 0:1], AF.Sqrt)
        ones_row = cpool.tile([1, P], f16)       # lhsT of the y2-row matmul
        nc.gpsimd.memset(ones_row[:], 1.0)
        ones_col = cpool.tile([P, 1], bf16)      # lhsT of the y2 column-reduce
        nc.gpsimd.memset(ones_col[:], 1.0)
        beta_b = cpool.tile([P, 1], f32)
        b11 = cpool.tile([1, 1], f32)

        xdt = f8 if use_fp8 else bf16
        x2 = cpool.tile([P, mt], f32)
        xsq = cpool.tile([P, rank], f32)
        xT8 = cpool.tile([P, kc, rows], xdt)
        y8 = cpool.tile([P, kc, ncol], xdt)
        e_row = cpool.tile([1, ncol], f16)
        xs_sb = cpool.tile([P, mt, rank], f32)
        y_ap = y_d.ap().rearrange("(c p) n -> p c n", p=P)

        # ---- loads: Y chunk 0 first (it gates the first e_row chain),
        # then X; later Y chunks prefetched inside the pipeline.
        yf_tiles = {}

        def load_chunk(i):
            yfi = yf_pool.tile([P, kc, gw], f32, name="yf", tag="yf")
            nc.sync.dma_start(yfi[:], y_ap[:, :, i * gw : (i + 1) * gw])
            yf_tiles[i] = yfi

        # load order: Xa (m-tiles 0-3), Y0 halves, Xb, beta -- the first
        # octet's m=0 chain needs only Xa + Y0, so the serial DMA prefix
        # ahead of the first sqrt shrinks to ~8us.
        mh = mt // 2
        nc.sync.dma_start(
            xs_sb[:, 0:mh, :],
            xs_d.ap()[0 : mh * P, :].rearrange("(t p) k -> p t k", p=P))
        hw_ = gw // 2
        yf0h = []
        for h in range(2):
            t = yf_pool.tile([P, kc, hw_], f32, name="yfh", tag="yfh")
            nc.sync.dma_start(t[:], y_ap[:, :, h * hw_ : (h + 1) * hw_])
            yf0h.append(t)
        nc.sync.dma_start(
            xs_sb[:, mh:mt, :],
            xs_d.ap()[mh * P : mt * P, :].rearrange("(t p) k -> p t k", p=P))
        nc.sync.dma_start(b11[:], beta_d.ap()[:])
        nc.gpsimd.partition_broadcast(beta_b[:], b11[:])

        # ---- X side ----
        tp_psum_ctx = ExitStack()
        tp_psum = tp_psum_ctx.enter_context(
            tc.tile_pool(name="tpp", bufs=2, space="PSUM"))
        # 4 transposes (c-major m-pairs) per PSUM bank -> one batched
        # -2x fp8 convert per bank (issued BEFORE the x2 squares so xT8 is
        # ready for the first DR matmul as early as possible)
        for mp in range(mt // 2):
            m0 = mp * 2
            pt = tp_psum.tile([P, 2, 2, P], f32)
            for c in range(kc):
                for dm in range(2):
                    nc.tensor.transpose(
                        pt[:, c, dm], xs_sb[:, m0 + dm, c * P : (c + 1) * P],
                        identity[:])
            dst = xT8[:, :, m0 * P : (m0 + 2) * P]
            if scales_on == "act":
                nc.scalar.activation(dst, pt[:], AF.Copy, scale=-2.0)
            elif scales_on == "vec":
                nc.vector.tensor_scalar(dst, pt[:], -2.0, None, op0=ALU.mult)
            else:
                nc.gpsimd.tensor_scalar(dst, pt[:], -2.0, None, op0=ALU.mult)
        tp_psum_ctx.close()
        if x2_on == "act":
            for m in range(mt):
                nc.scalar.activation(
                    xsq[:, 0:256], xs_sb[:, m, :], AF.Square,
                    accum_out=x2[:, m : m + 1])

        mm_psum = ctx.enter_context(
            tc.tile_pool(name="mmp", bufs=psum_bufs, space="PSUM"))

        ysq_tiles = {}

        def process_pre(i):
            """fp8 convert + squares for Y chunk i (DVE/Pool only)."""
            j0 = i * gw
            yfi = yf_tiles.pop(i)
            nc.vector.tensor_copy(y8[:, :, j0 : j0 + gw], yfi[:])
            ysq = ysq_pool.tile([P, kc, gw], bf16)
            # squares split into column halves: Pool does b01, DVE b23
            h = gw // 2
            nc.gpsimd.tensor_tensor(
                ysq[:, :, 0:h], yfi[:, :, 0:h], yfi[:, :, 0:h], op=ALU.mult)
            nc.vector.tensor_tensor(
                ysq[:, :, h:gw], yfi[:, :, h:gw], yfi[:, :, h:gw],
                op=ALU.mult)
            ysq_tiles[i] = ysq

        def process_post(i):
            """y2 column reduce (PE) + e_row copies (Pool) for chunk i."""
            j0 = i * gw
            ysq = ysq_tiles.pop(i)
            y2scr = mm_psum.tile([P, gw], f32, name="y2scr", tag="mm")
            for b in range(qn):
                sl = y2scr[0:1, b * FN : (b + 1) * FN]
                for c in range(kc):
                    nc.tensor.matmul(
                        sl, ones_col[:], ysq[:, c, b * FN : (b + 1) * FN],
                        start=(c == 0), stop=(c == kc - 1))
                nc.vector.tensor_copy(
                    e_row[:, j0 + b * FN : j0 + (b + 1) * FN], sl)

        # ---- software-pipelined chunk loop. chunk i+1's DMA + DVE/Pool
        # processing are issued before octet i (they don't touch the PE),
        # but its y2-reduce matmuls go AFTER octet i so the in-order PE
        # queue never stalls on not-yet-loaded Y data.
        # chunk 0 arrives as two 1024-wide halves -> first e_row sooner
        ysq0 = ysq_pool.tile([P, kc, gw], bf16, name="ysq", tag="ysq")
        y2scr0 = mm_psum.tile([P, gw], f32, name="y2scr", tag="mm")
        q_ = hw_ // 2
        for h in range(2):
            t = yf0h[h]
            nc.vector.tensor_copy(y8[:, :, h * hw_ : (h + 1) * hw_], t[:])
            nc.gpsimd.tensor_tensor(
                ysq0[:, :, h * hw_ : h * hw_ + q_],
                t[:, :, 0:q_], t[:, :, 0:q_], op=ALU.mult)
            nc.vector.tensor_tensor(
                ysq0[:, :, h * hw_ + q_ : (h + 1) * hw_],
                t[:, :, q_:hw_], t[:, :, q_:hw_], op=ALU.mult)
            for b in ((0, 1) if h == 0 else (2, 3)):
                sl = y2scr0[0:1, b * FN : (b + 1) * FN]
                for c in range(kc):
                    nc.tensor.matmul(
                        sl, ones_col[:], ysq0[:, c, b * FN : (b + 1) * FN],
                        start=(c == 0), stop=(c == kc - 1))
                nc.vector.tensor_copy(
                    e_row[:, b * FN : (b + 1) * FN], sl)
        for i in range(ng):
            j0 = i * gw
            if i + 1 < ng:
                load_chunk(i + 1)
                process_pre(i + 1)
            for m in range(mt):
                ps = mm_psum.tile([P, gw], f32, name="ps", tag="mm")
                for q in range(qn):
                    jq = j0 + q * FN
                    sl = ps[:, q * FN : (q + 1) * FN]
                    if use_fp8:
                        nc.tensor.matmul(
                            sl, xT8[:, :, m * P : (m + 1) * P],
                            y8[:, :, jq : jq + FN],
                            start=True, stop=False, perf_mode=DR)
                    else:
                        for c in range(kc):
                            nc.tensor.matmul(
                                sl, xT8[:, c, m * P : (m + 1) * P],
                                y8[:, c, jq : jq + FN],
                                start=(c == 0), stop=False)
                    nc.tensor.matmul(
                        sl, ones_row[:], e_row[:, jq : jq + FN],
                        start=False, stop=True)
                d = dpool.tile([P, gw], f16)
                nc.scalar.activation(
                    d[:], ps[:], AF.Sqrt, bias=x2[:, m : m + 1])
                nc.vector.tensor_scalar(
                    d[:], d[:], -1.0, beta_b[:], op0=ALU.mult, op1=ALU.add)
                blk = (m * ng + i) * P
                nc.sync.dma_start(z_d.ap()[blk : blk + P, :], d[:])
                if m == min(post_at, mt - 1) and i + 1 < ng:
                    process_post(i + 1)
                del d

    nc.compile()
    return nc


_CACHED = {}


def _get_nc():
    if "nc" not in _CACHED:
        _CACHED["nc"] = build_l2_kernel()
    return _CACHED["nc"]


def kernel(X, Y, beta):
    X = np.ascontiguousarray(np.asarray(X, dtype=np.float32))
    Y = np.ascontiguousarray(np.asarray(Y, dtype=np.float32))
    beta = np.asarray(beta, dtype=np.float32).reshape(1, 1)
    assert X.shape == (N_ROW, RANK) and Y.shape == (RANK, N_COL)

    nc = _get_nc()
    in_maps = [
        {"Xs": X[c * ROWS_PER_CORE : (c + 1) * ROWS_PER_CORE], "Y": Y,
         "beta": beta}
        for c in range(N_CORES)
    ]
    res = run_bass_kernel_spmd(nc, in_maps, core_ids=list(range(N_CORES)))
    mt, ng = ROWS_PER_CORE // P, N_COL // GW
    out = np.empty((N_ROW, N_COL), dtype=np.float32)
    for c in range(N_CORES):
        slab = res.results[c]["Z"].reshape(mt, ng, P, GW)
        slab = slab.transpose(0, 2, 1, 3).reshape(ROWS_PER_CORE, N_COL)
        out[c * ROWS_PER_CORE : (c + 1) * ROWS_PER_CORE] = slab
    return out


# revision 5
# speedup vs baseline: 6.9946x; 1.0495x over previous
"""Pairwise L2-distance kernel (retrieval_knn) for 8x Trainium2 NeuronCores.

Z = beta - sqrt(max(||x||^2 + ||y||^2 - 2 X@Y, 0)),
X:(8192,256) f32, Y:(256,8192) f32, beta:(1,) -> Z:(8192,8192) f32.
X row-sharded over 8 cores; Y/beta replicated.

Structure (per core):
  - fp8e4 DoubleRow cross matmuls: K=256 in one PE instruction per
    512-wide PSUM quarter; ||y||^2 added via one fp16 ones-row matmul.
  - software-pipelined chunk loop: {DMA Y chunk -> fp8 convert (DVE),
    squares (Pool), y2 column-reduce (PE, into an mm-psum scratch tile),
    e_row copy (DVE)} immediately followed by that chunk's 8 m-groups:
    {matmuls -> ScalarE sqrt(ps + x2) -> f16, DVE (d*-1)+beta in f16,
    256KB f16 block store}.
  - ScalarE runs only x-side prep + the 32 sqrt groups (2048 wide, 4 PSUM
    banks, double buffered); everything else lives on DVE/Pool/PE.
  - output f16, upcast to f32 on the host.
"""

from contextlib import ExitStack

import numpy as np

import concourse.bacc as bacc
import concourse.mybir as mybir
import concourse.tile as tile
from concourse.bass_utils import run_bass_kernel_spmd
from concourse.masks import make_identity

N_CORES = 8
N_ROW, RANK, N_COL = 8192, 256, 8192
ROWS_PER_CORE = N_ROW // N_CORES  # 1024

P = 128      # partitions
FN = 512     # matmul free dim / PSUM bank (fp32)
GW = 2048    # ACT/DVE group width (4 PSUM banks) and Y chunk width

f32 = mybir.dt.float32
bf16 = mybir.dt.bfloat16
f16 = mybir.dt.float16
f8 = mybir.dt.float8e4

AF = mybir.ActivationFunctionType
ALU = mybir.AluOpType
DR = mybir.MatmulPerfMode.DoubleRow


def build_l2_kernel(rows=ROWS_PER_CORE, rank=RANK, ncol=N_COL, n_cores=N_CORES,
                    use_fp8=True, gw=GW, d_bufs=8, psum_bufs=2, yf_bufs=4, post_at=3,
                    x2_on="act", scales_on="act", split_sq0=True):
    """Build the per-core SPMD Bass program. Returns the compiled Bacc."""
    kc = rank // P          # k-chunks (2)
    mt = rows // P          # m-tiles (8)
    ng = ncol // gw         # chunks == main-loop column groups (4)
    qn = gw // FN           # 512-quarters per group (4)
    assert rows % P == 0 and rank == 2 * P and ncol % gw == 0 and gw % FN == 0

    nc = bacc.Bacc("TRN2", target_bir_lowering=False, debug=False,
                   num_devices=n_cores)

    xs_d = nc.dram_tensor("Xs", [rows, rank], f32, kind="ExternalInput")
    y_d = nc.dram_tensor("Y", [rank, ncol], f32, kind="ExternalInput")
    beta_d = nc.dram_tensor("beta", [1, 1], f32, kind="ExternalInput")
    # Z stored as [m, g, 128, gw] f16 blocks; host reassembles + upcasts.
    z_d = nc.dram_tensor("Z", [mt * ng * P, gw], f16, kind="ExternalOutput")

    with tile.TileContext(nc) as tc, ExitStack() as ctx:
        cpool = ctx.enter_context(tc.tile_pool(name="const", bufs=1))
        yf_pool = ctx.enter_context(tc.tile_pool(name="yf", bufs=yf_bufs))
        ysq_pool = ctx.enter_context(tc.tile_pool(name="ysq", bufs=2))
        dpool = ctx.enter_context(tc.tile_pool(name="d", bufs=d_bufs))

        # ---- constants ----
        identity = cpool.tile([P, P], f32)
        make_identity(nc, identity[:])
        warm = cpool.tile([1, 1], f32)
        nc.scalar.activation(warm[:], identity[0:1, 0:1], AF.Sqrt)
        ones_row = cpool.tile([1, P], f16)       # lhsT of the y2-row matmul
        nc.gpsimd.memset(ones_row[:], 1.0)
        ones_col = cpool.tile([P, 1], bf16)      # lhsT of the y2 column-reduce
        nc.gpsimd.memset(ones_col[:], 1.0)
        beta_b = cpool.tile([P, 1], f32)
        b11 = cpool.tile([1, 1], f32)

        xdt = f8 if use_fp8 else bf16
        x2 = cpool.tile([P, mt], f32)
        xsq = cpool.tile([P, rank], f32)
        xT8 = cpool.tile([P, kc, rows], xdt)
        y8 = cpool.tile([P, kc, ncol], xdt)
        e_row = cpool.tile([1, ncol], f16)
        ar_pool = ctx.enter_context(tc.tile_pool(name="ar", bufs=2))
        xs_sb = cpool.tile([P, mt, rank], f32)
        y_ap = y_d.ap().rearrange("(c p) n -> p c n", p=P)

        # ---- loads: Y chunk 0 first (it gates the first e_row chain),
        # then X; later Y chunks prefetched inside the pipeline.
        yf_tiles = {}

        def load_chunk(i):
            yfi = yf_pool.tile([P, kc, gw], f32, name="yf", tag="yf")
            nc.sync.dma_start(yfi[:], y_ap[:, :, i * gw : (i + 1) * gw])
            yf_tiles[i] = yfi

        # load order: Xa (m-tiles 0-3), Y0 halves, Xb, beta -- the first
        # octet's m=0 chain needs only Xa + Y0, so the serial DMA prefix
        # ahead of the first sqrt shrinks to ~8us.
        mh = mt // 2
        nc.sync.dma_start(
            xs_sb[:, 0:mh, :],
            xs_d.ap()[0 : mh * P, :].rearrange("(t p) k -> p t k", p=P))
        hw_ = gw // 2
        yf0h = []
        for h in range(2):
            t = yf_pool.tile([P, kc, hw_], f32, name="yfh", tag="yfh")
            nc.sync.dma_start(t[:], y_ap[:, :, h * hw_ : (h + 1) * hw_])
            yf0h.append(t)
        nc.sync.dma_start(
            xs_sb[:, mh:mt, :],
            xs_d.ap()[mh * P : mt * P, :].rearrange("(t p) k -> p t k", p=P))
        nc.sync.dma_start(b11[:], beta_d.ap()[:])
        nc.gpsimd.partition_broadcast(beta_b[:], b11[:])
        for _i in range(1, ng):
            load_chunk(_i)

        # ---- X side ----
        tp_psum_ctx = ExitStack()
        tp_psum = tp_psum_ctx.enter_context(
            tc.tile_pool(name="tpp", bufs=2, space="PSUM"))
        # 4 transposes (c-major m-pairs) per PSUM bank -> one batched
        # -2x fp8 convert per bank (issued BEFORE the x2 squares so xT8 is
        # ready for the first DR matmul as early as possible)
        for mp in range(mt // 2):
            m0 = mp * 2
            pt = tp_psum.tile([P, 2, 2, P], f32)
            for c in range(kc):
                for dm in range(2):
                    nc.tensor.transpose(
                        pt[:, c, dm], xs_sb[:, m0 + dm, c * P : (c + 1) * P],
                        identity[:])
            dst = xT8[:, :, m0 * P : (m0 + 2) * P]
            if scales_on == "act":
                nc.scalar.activation(dst, pt[:], AF.Copy, scale=-2.0)
            elif scales_on == "vec":
                nc.vector.tensor_scalar(dst, pt[:], -2.0, None, op0=ALU.mult)
            else:
                nc.gpsimd.tensor_scalar(dst, pt[:], -2.0, None, op0=ALU.mult)
        tp_psum_ctx.close()
        if x2_on == "act":
            for m in range(mt):
                nc.scalar.activation(
                    xsq[:, 0:256], xs_sb[:, m, :], AF.Square,
                    accum_out=x2[:, m : m + 1])

        mm_psum = ctx.enter_context(
            tc.tile_pool(name="mmp", bufs=psum_bufs, space="PSUM"))

        ysq_tiles = {}

        def process_pre(i):
            """fp8 convert + squares for Y chunk i (DVE/Pool only)."""
            j0 = i * gw
            yfi = yf_tiles.pop(i)
            nc.vector.tensor_copy(y8[:, :, j0 : j0 + gw], yfi[:])
            ysq = ysq_pool.tile([P, kc, gw], bf16)
            # squares split into column halves: Pool does b01, DVE b23
            h = gw // 2
            nc.gpsimd.tensor_tensor(
                ysq[:, :, 0:h], yfi[:, :, 0:h], yfi[:, :, 0:h], op=ALU.mult)
            nc.vector.tensor_tensor(
                ysq[:, :, h:gw], yfi[:, :, h:gw], yfi[:, :, h:gw],
                op=ALU.mult)
            ysq_tiles[i] = ysq

        def process_post(i):
            """y2 reduce for chunk i: two half-width all_reduces on Pool."""
            j0 = i * gw
            ysq = ysq_tiles.pop(i)
            for h in range(2):
                y2_reduce(ysq[:, :, h * hw_ : (h + 1) * hw_],
                          j0 + h * hw_, hw_)

        # ---- software-pipelined chunk loop. chunk i+1's DMA + DVE/Pool
        # processing are issued before octet i (they don't touch the PE),
        # but its y2-reduce matmuls go AFTER octet i so the in-order PE
        # queue never stalls on not-yet-loaded Y data.
        import concourse.bass_isa as bass_isa

        def y2_reduce(ysq_ap, j0, w):
            """e_row[j0:j0+w] = sum over 256 k of ysq via gpsimd
            partition_all_reduce (no PSUM, no PE, no DVE copies)."""
            ar = ar_pool.tile([P, kc * (gw // 2)], f32, name="ar", tag="ar")
            a = ar[:, 0 : kc * w]
            nc.gpsimd.partition_all_reduce(
                a, ysq_ap, channels=P, reduce_op=bass_isa.ReduceOp.add)
            nc.vector.tensor_tensor(
                e_row[:, j0 : j0 + w], a[0:1, 0:w], a[0:1, w : 2 * w],
                op=ALU.add)

        # chunk 0 arrives as two 1024-wide halves -> first e_row sooner
        ysq0 = ysq_pool.tile([P, kc, gw], bf16, name="ysq", tag="ysq")
        q_ = hw_ // 2
        for h in range(2):
            t = yf0h[h]
            nc.vector.tensor_copy(y8[:, :, h * hw_ : (h + 1) * hw_], t[:])
            nc.gpsimd.tensor_tensor(
                ysq0[:, :, h * hw_ : h * hw_ + q_],
                t[:, :, 0:q_], t[:, :, 0:q_], op=ALU.mult)
            nc.vector.tensor_tensor(
                ysq0[:, :, h * hw_ + q_ : (h + 1) * hw_],
                t[:, :, q_:hw_], t[:, :, q_:hw_], op=ALU.mult)
            y2_reduce(ysq0[:, :, h * hw_ : (h + 1) * hw_], h * hw_, hw_)
        for i in range(ng):
            j0 = i * gw
            if i + 1 < ng:
                process_pre(i + 1)
            for m in range(mt):
                ps = mm_psum.tile([P, gw], f32, name="ps", tag="mm")
                for q in range(qn):
                    jq = j0 + q * FN
                    sl = ps[:, q * FN : (q + 1) * FN]
                    if use_fp8:
                        nc.tensor.matmul(
                            sl, xT8[:, :, m * P : (m + 1) * P],
                            y8[:, :, jq : jq + FN],
                            start=True, stop=False, perf_mode=DR)
                    else:
                        for c in range(kc):
                            nc.tensor.matmul(
                                sl, xT8[:, c, m * P : (m + 1) * P],
                                y8[:, c, jq : jq + FN],
                                start=(c == 0), stop=False)
                    nc.tensor.matmul(
                        sl, ones_row[:], e_row[:, jq : jq + FN],
                        start=False, stop=True)
                d = dpool.tile([P, gw], f16)
                nc.scalar.activation(
                    d[:], ps[:], AF.Sqrt, bias=x2[:, m : m + 1])
                nc.vector.tensor_scalar(
                    d[:], d[:], -1.0, beta_b[:], op0=ALU.mult, op1=ALU.add)
                blk = (m * ng + i) * P
                nc.sync.dma_start(z_d.ap()[blk : blk + P, :], d[:])
                if m == min(post_at, mt - 1) and i + 1 < ng:
                    process_post(i + 1)
                del d

    nc.compile()
    return nc


_CACHED = {}


def _get_nc():
    if "nc" not in _CACHED:
        _CACHED["nc"] = build_l2_kernel()
    return _CACHED["nc"]


def kernel(X, Y, beta):
    X = np.ascontiguousarray(np.asarray(X, dtype=np.float32))
    Y = np.ascontiguousarray(np.asarray(Y, dtype=np.float32))
    beta = np.asarray(beta, dtype=np.float32).reshape(1, 1)
    assert X.shape == (N_ROW, RANK) and Y.shape == (RANK, N_COL)

    nc = _get_nc()
    in_maps = [
        {"Xs": X[c * ROWS_PER_CORE : (c + 1) * ROWS_PER_CORE], "Y": Y,
         "beta": beta}
        for c in range(N_CORES)
    ]
    res = run_bass_kernel_spmd(nc, in_maps, core_ids=list(range(N_CORES)))
    mt, ng = ROWS_PER_CORE // P, N_COL // GW
    out = np.empty((N_ROW, N_COL), dtype=np.float32)
    for c in range(N_CORES):
        slab = res.results[c]["Z"].reshape(mt, ng, P, GW)
        slab = slab.transpose(0, 2, 1, 3).reshape(ROWS_PER_CORE, N_COL)
        out[c * ROWS_PER_CORE : (c + 1) * ROWS_PER_CORE] = slab
    return out


# revision 6
# speedup vs baseline: 7.1827x; 1.0269x over previous
"""Pairwise L2-distance kernel (retrieval_knn) for 8x Trainium2 NeuronCores.

Z = beta - sqrt(max(||x||^2 + ||y||^2 - 2 X@Y, 0)),
X:(8192,256) f32, Y:(256,8192) f32, beta:(1,) -> Z:(8192,8192) f32.
X row-sharded over 8 cores; Y/beta replicated.

Structure (per core):
  - fp8e4 DoubleRow cross matmuls: K=256 in one PE instruction per
    512-wide PSUM quarter; ||y||^2 added via one fp16 ones-row matmul.
  - software-pipelined chunk loop: {DMA Y chunk -> fp8 convert (DVE),
    squares (Pool), y2 column-reduce (PE, into an mm-psum scratch tile),
    e_row copy (DVE)} immediately followed by that chunk's 8 m-groups:
    {matmuls -> ScalarE sqrt(ps + x2) -> f16, DVE (d*-1)+beta in f16,
    256KB f16 block store}.
  - ScalarE runs only x-side prep + the 32 sqrt groups (2048 wide, 4 PSUM
    banks, double buffered); everything else lives on DVE/Pool/PE.
  - output f16, upcast to f32 on the host.
"""

from contextlib import ExitStack

import numpy as np

import concourse.bacc as bacc
import concourse.mybir as mybir
import concourse.tile as tile
from concourse.bass_utils import run_bass_kernel_spmd
from concourse.masks import make_identity

N_CORES = 8
N_ROW, RANK, N_COL = 8192, 256, 8192
ROWS_PER_CORE = N_ROW // N_CORES  # 1024

P = 128      # partitions
FN = 512     # matmul free dim / PSUM bank (fp32)
GW = 2048    # ACT/DVE group width (4 PSUM banks) and Y chunk width

f32 = mybir.dt.float32
bf16 = mybir.dt.bfloat16
f16 = mybir.dt.float16
f8 = mybir.dt.float8e4

AF = mybir.ActivationFunctionType
ALU = mybir.AluOpType
DR = mybir.MatmulPerfMode.DoubleRow


def build_l2_kernel(rows=ROWS_PER_CORE, rank=RANK, ncol=N_COL, n_cores=N_CORES,
                    use_fp8=True, gw=GW, d_bufs=8, psum_bufs=2, yf_bufs=4, post_at=3,
                    x2_on="act", scales_on="act", split_sq0=True):
    """Build the per-core SPMD Bass program. Returns the compiled Bacc."""
    kc = rank // P          # k-chunks (2)
    mt = rows // P          # m-tiles (8)
    ng = ncol // gw         # chunks == main-loop column groups (4)
    qn = gw // FN           # 512-quarters per group (4)
    assert rows % P == 0 and rank == 2 * P and ncol % gw == 0 and gw % FN == 0

    nc = bacc.Bacc("TRN2", target_bir_lowering=False, debug=False,
                   num_devices=n_cores)

    xs_d = nc.dram_tensor("Xs", [rows, rank], f32, kind="ExternalInput")
    y_d = nc.dram_tensor("Y", [rank, ncol], f32, kind="ExternalInput")
    beta_d = nc.dram_tensor("beta", [1, 1], f32, kind="ExternalInput")
    # Z stored as [m, g, 128, gw] f16 blocks; host reassembles + upcasts.
    z_d = nc.dram_tensor("Z", [mt * ng * P, gw], f16, kind="ExternalOutput")

    with tile.TileContext(nc) as tc, ExitStack() as ctx:
        cpool = ctx.enter_context(tc.tile_pool(name="const", bufs=1))
        yf_pool = ctx.enter_context(tc.tile_pool(name="yf", bufs=yf_bufs))
        ysq_pool = ctx.enter_context(tc.tile_pool(name="ysq", bufs=2))
        dpool = ctx.enter_context(tc.tile_pool(name="d", bufs=d_bufs))

        # ---- constants ----
        identity = cpool.tile([P, P], f32)
        make_identity(nc, identity[:])
        warm = cpool.tile([1, 1], f32)
        nc.scalar.activation(warm[:], identity[0:1, 0:1], AF.Sqrt)
        ones_row = cpool.tile([1, P], f16)       # lhsT of the y2-row matmul
        nc.gpsimd.memset(ones_row[:], 1.0)
        ones_col = cpool.tile([P, 1], bf16)      # lhsT of the y2 column-reduce
        nc.gpsimd.memset(ones_col[:], 1.0)
        beta_b = cpool.tile([P, 1], f32)
        b11 = cpool.tile([1, 1], f32)

        xdt = f8 if use_fp8 else bf16
        x2 = cpool.tile([P, mt], f32)
        xsq = cpool.tile([P, rank], f32)
        xT8 = cpool.tile([P, kc, rows], xdt)
        y8 = cpool.tile([P, kc, ncol], xdt)
        e_row = cpool.tile([1, ncol], f16)
        ar_pool = ctx.enter_context(tc.tile_pool(name="ar", bufs=2))
        xs_sb = cpool.tile([P, mt, rank], f32)
        y_ap = y_d.ap().rearrange("(c p) n -> p c n", p=P)

        # ---- loads: Y chunk 0 first (it gates the first e_row chain),
        # then X; later Y chunks prefetched inside the pipeline.
        yf_tiles = {}

        def load_chunk(i):
            yfi = yf_pool.tile([P, kc, gw], f32, name="yf", tag="yf")
            nc.sync.dma_start(yfi[:], y_ap[:, :, i * gw : (i + 1) * gw])
            yf_tiles[i] = yfi

        # load order: Xa (m-tiles 0-3), Y0 halves, Xb, beta -- the first
        # octet's m=0 chain needs only Xa + Y0, so the serial DMA prefix
        # ahead of the first sqrt shrinks to ~8us.
        mh = mt // 2
        nc.sync.dma_start(
            xs_sb[:, 0:mh, :],
            xs_d.ap()[0 : mh * P, :].rearrange("(t p) k -> p t k", p=P))
        hw_ = gw // 2
        yf0h = []
        for h in range(2):
            t = yf_pool.tile([P, kc, hw_], f32, name="yfh", tag="yfh")
            nc.sync.dma_start(t[:], y_ap[:, :, h * hw_ : (h + 1) * hw_])
            yf0h.append(t)
        nc.sync.dma_start(
            xs_sb[:, mh:mt, :],
            xs_d.ap()[mh * P : mt * P, :].rearrange("(t p) k -> p t k", p=P))
        nc.sync.dma_start(b11[:], beta_d.ap()[:])
        nc.gpsimd.partition_broadcast(beta_b[:], b11[:])
        for _i in range(1, ng):
            load_chunk(_i)

        # ---- X side (issued in halves, interleaved with chunk-0 work in
        # data-arrival order: Xa-dependent first, Xb-dependent later) ----
        tp_psum_ctx = ExitStack()
        tp_psum = tp_psum_ctx.enter_context(
            tc.tile_pool(name="tpp", bufs=2, space="PSUM"))

        def x_side(mp_lo, mp_hi):
            for mp in range(mp_lo, mp_hi):
                m0 = mp * 2
                pt = tp_psum.tile([P, 2, 2, P], f32, name="pt", tag="pt")
                for c in range(kc):
                    for dm in range(2):
                        nc.tensor.transpose(
                            pt[:, c, dm],
                            xs_sb[:, m0 + dm, c * P : (c + 1) * P],
                            identity[:])
                dst = xT8[:, :, m0 * P : (m0 + 2) * P]
                nc.scalar.activation(dst, pt[:], AF.Copy, scale=-2.0)
                for dm in range(2):
                    nc.scalar.activation(
                        xsq[:, 0:256], xs_sb[:, m0 + dm, :], AF.Square,
                        accum_out=x2[:, m0 + dm : m0 + dm + 1])


        ysq_tiles = {}

        def process_pre(i):
            """fp8 convert + squares for Y chunk i (DVE/Pool only)."""
            j0 = i * gw
            yfi = yf_tiles.pop(i)
            nc.vector.tensor_copy(y8[:, :, j0 : j0 + gw], yfi[:])
            ysq = ysq_pool.tile([P, kc, gw], bf16)
            # squares split into column halves: Pool does b01, DVE b23
            h = gw // 2
            nc.gpsimd.tensor_tensor(
                ysq[:, :, 0:h], yfi[:, :, 0:h], yfi[:, :, 0:h], op=ALU.mult)
            nc.vector.tensor_tensor(
                ysq[:, :, h:gw], yfi[:, :, h:gw], yfi[:, :, h:gw],
                op=ALU.mult)
            ysq_tiles[i] = ysq

        def process_post(i):
            """y2 reduce for chunk i: two half-width all_reduces on Pool."""
            j0 = i * gw
            ysq = ysq_tiles.pop(i)
            for h in range(2):
                y2_reduce(ysq[:, :, h * hw_ : (h + 1) * hw_],
                          j0 + h * hw_, hw_)

        # ---- software-pipelined chunk loop. chunk i+1's DMA + DVE/Pool
        # processing are issued before octet i (they don't touch the PE),
        # but its y2-reduce matmuls go AFTER octet i so the in-order PE
        # queue never stalls on not-yet-loaded Y data.
        import concourse.bass_isa as bass_isa

        def y2_reduce(ysq_ap, j0, w):
            """e_row[j0:j0+w] = sum over 256 k of ysq via gpsimd
            partition_all_reduce (no PSUM, no PE, no DVE copies)."""
            ar = ar_pool.tile([P, kc * (gw // 2)], f32, name="ar", tag="ar")
            a = ar[:, 0 : kc * w]
            nc.gpsimd.partition_all_reduce(
                a, ysq_ap, channels=P, reduce_op=bass_isa.ReduceOp.add)
            nc.vector.tensor_tensor(
                e_row[:, j0 : j0 + w], a[0:1, 0:w], a[0:1, w : 2 * w],
                op=ALU.add)

        # chunk 0 arrives as two 1024-wide halves; its y2 goes through the
        # LOW-LATENCY PE-matmul path into a pre-main PSUM scratch (the Pool
        # all_reduce is serial ~10us behind the squares, which would gate
        # the very first sqrt group; chunks 1-3 keep the all_reduce).
        ysq0 = ysq_pool.tile([P, kc, gw], bf16, name="ysq", tag="ysq")
        y2s0_ctx = ExitStack()
        y2s0_pool = y2s0_ctx.enter_context(
            tc.tile_pool(name="y2s0", bufs=1, space="PSUM"))
        y2scr0 = y2s0_pool.tile([1, gw], f32)
        q_ = hw_ // 2
        for h in range(2):
            x_side(h * (mt // 4), (h + 1) * (mt // 4) if h == 0 else mt // 2)
            t = yf0h[h]
            nc.vector.tensor_copy(y8[:, :, h * hw_ : (h + 1) * hw_], t[:])
            nc.gpsimd.tensor_tensor(
                ysq0[:, :, h * hw_ : h * hw_ + q_],
                t[:, :, 0:q_], t[:, :, 0:q_], op=ALU.mult)
            nc.vector.tensor_tensor(
                ysq0[:, :, h * hw_ + q_ : (h + 1) * hw_],
                t[:, :, q_:hw_], t[:, :, q_:hw_], op=ALU.mult)
            for b in ((0, 1) if h == 0 else (2, 3)):
                sl = y2scr0[0:1, b * FN : (b + 1) * FN]
                for c in range(kc):
                    nc.tensor.matmul(
                        sl, ones_col[:],
                        ysq0[:, c, b * FN : (b + 1) * FN],
                        start=(c == 0), stop=(c == kc - 1))
                nc.vector.tensor_copy(e_row[:, b * FN : (b + 1) * FN], sl)
        y2s0_ctx.close()
        tp_psum_ctx.close()
        mm_psum = ctx.enter_context(
            tc.tile_pool(name="mmp", bufs=psum_bufs, space="PSUM"))
        for i in range(ng):
            j0 = i * gw
            if i + 1 < ng:
                process_pre(i + 1)
            for m in range(mt):
                ps = mm_psum.tile([P, gw], f32, name="ps", tag="mm")
                for q in range(qn):
                    jq = j0 + q * FN
                    sl = ps[:, q * FN : (q + 1) * FN]
                    if use_fp8:
                        nc.tensor.matmul(
                            sl, xT8[:, :, m * P : (m + 1) * P],
                            y8[:, :, jq : jq + FN],
                            start=True, stop=False, perf_mode=DR)
                    else:
                        for c in range(kc):
                            nc.tensor.matmul(
                                sl, xT8[:, c, m * P : (m + 1) * P],
                                y8[:, c, jq : jq + FN],
                                start=(c == 0), stop=False)
                    nc.tensor.matmul(
                        sl, ones_row[:], e_row[:, jq : jq + FN],
                        start=False, stop=True)
                d = dpool.tile([P, gw], f16)
                nc.scalar.activation(
                    d[:], ps[:], AF.Sqrt, bias=x2[:, m : m + 1])
                nc.vector.tensor_scalar(
                    d[:], d[:], -1.0, beta_b[:], op0=ALU.mult, op1=ALU.add)
                blk = (m * ng + i) * P
                nc.sync.dma_start(z_d.ap()[blk : blk + P, :], d[:])
                if m == min(post_at, mt - 1) and i + 1 < ng:
                    process_post(i + 1)
                del d

    nc.compile()
    return nc


_CACHED = {}


def _get_nc():
    if "nc" not in _CACHED:
        _CACHED["nc"] = build_l2_kernel()
    return _CACHED["nc"]


def kernel(X, Y, beta):
    X = np.ascontiguousarray(np.asarray(X, dtype=np.float32))
    Y = np.ascontiguousarray(np.asarray(Y, dtype=np.float32))
    beta = np.asarray(beta, dtype=np.float32).reshape(1, 1)
    assert X.shape == (N_ROW, RANK) and Y.shape == (RANK, N_COL)

    nc = _get_nc()
    in_maps = [
        {"Xs": X[c * ROWS_PER_CORE : (c + 1) * ROWS_PER_CORE], "Y": Y,
         "beta": beta}
        for c in range(N_CORES)
    ]
    res = run_bass_kernel_spmd(nc, in_maps, core_ids=list(range(N_CORES)))
    mt, ng = ROWS_PER_CORE // P, N_COL // GW
    out = np.empty((N_ROW, N_COL), dtype=np.float32)
    for c in range(N_CORES):
        slab = res.results[c]["Z"].reshape(mt, ng, P, GW)
        slab = slab.transpose(0, 2, 1, 3).reshape(ROWS_PER_CORE, N_COL)
        out[c * ROWS_PER_CORE : (c + 1) * ROWS_PER_CORE] = slab
    return out
